# revision 55
# baseline (speedup 1.0000x reference)
"""Llama4 MoE (T=1024, H=1024, I=2048, SI=4096, E=8, K=1) on 8 trn2 NeuronCores.

Sharding (expert-parallel + shared-TP, host-side combine):
  - core c owns expert c (full gate/up/down) plus a 512-wide slice of the
    shared expert. Every core routes all tokens (cheap), compacts its
    expert's tokens into C capacity slots, runs the expert MLP at width C,
    and scatter-adds the result back to token rows.
  - Host: out = sum_c (shared_partial_c + routed_c).

Everything runs in bf16 on the PE (1 cycle/row at any free size; halves HBM
traffic vs fp32). The router is computed as xb@wb + xb@wr + xr@wb where
xb/wb are bf16 roundings and xr/wr bf16 residuals: max logit error ~2e-5
vs fp32, far below the minimum top-2 logit gap (~3e-4), so the argmax
matches the fp32 reference exactly.

Token dispatch uses the SWDGE DMA-gather (transpose mode): it gathers the
selected token rows from DRAM and writes them already transposed as
[h_part, ho, slot] -- zero tensor-engine cost. The return scatter uses the
SWDGE DMA scatter-add into a zero-initialized DRAM output (entry list is
shifted by 16 sacrificial entries because the scatter drops entry 0; row 0
of the +1-shifted output is the trash row).

Capacity C is chosen at runtime from the actual expert loads (host numpy
router), rounded up; the compiled program is cached per C.
"""

import functools
import os
import numpy as np
import ml_dtypes

BF = ml_dtypes.bfloat16

T, H, I, SI, E = 1024, 1024, 2048, 4096, 8
NCORES = 8
SIS = SI // NCORES  # 512
P = 128
HO = H // P         # 8
TT = T // P         # 8
IT = I // P         # 16
ST = SIS // P       # 4
NQ = 4              # token quarters for shared gate/up
QF = T // NQ        # 256
NH = 2              # token halves for shared down
NF = T // NH        # 512
CPAD = 256          # gather width (must be a multiple of 128)
BIG = 20000.0       # out-of-range slot for unselected tokens


def _build_nc(C):
    """C: expert token capacity (multiple of 16, <= CPAD-16)."""
    import concourse.mybir as mybir
    import concourse.tile as tile
    from concourse import bacc
    from concourse.masks import make_identity

    F32 = mybir.dt.float32
    BF16 = mybir.dt.bfloat16
    I16 = mybir.dt.int16
    AF = mybir.ActivationFunctionType
    ALU = mybir.AluOpType
    AX = mybir.AxisListType

    CE = C + 16   # scatter entries: first 16 sacrificial (entry-0 drop)
    CB = (CE + P - 1) // P       # entry partition-blocks for scatter source
    CI = CE // 16                # index columns used by the scatter

    nc = bacc.Bacc(trn_type="TRN2")

    xtb_d = nc.dram_tensor("xtb", [P, HO, T], BF16, kind="ExternalInput")
    xtr_d = nc.dram_tensor("xtr", [P, HO, T], BF16, kind="ExternalInput")
    xnat_d = nc.dram_tensor("xnat", [T, H], BF16, kind="ExternalInput")
    rwb_d = nc.dram_tensor("rwb", [P, HO, E], BF16, kind="ExternalInput")
    rwr_d = nc.dram_tensor("rwr", [P, HO, E], BF16, kind="ExternalInput")
    sgb_d = nc.dram_tensor("sgb", [P, HO, SIS], BF16, kind="ExternalInput")
    sub_d = nc.dram_tensor("sub", [P, HO, SIS], BF16, kind="ExternalInput")
    sdb_d = nc.dram_tensor("sdb", [P, ST, H], BF16, kind="ExternalInput")
    egb_d = nc.dram_tensor("egb", [P, HO, I], BF16, kind="ExternalInput")
    eub_d = nc.dram_tensor("eub", [P, HO, I], BF16, kind="ExternalInput")
    edb_d = nc.dram_tensor("edb", [P, IT, H], BF16, kind="ExternalInput")
    iotac_d = nc.dram_tensor("iotac", [P, CPAD], F32, kind="ExternalInput")
    iotag_d = nc.dram_tensor("iotag", [P, 16], F32, kind="ExternalInput")
    iotam_d = nc.dram_tensor("iotam", [P, P], F32, kind="ExternalInput")
    iotasm1_d = nc.dram_tensor("iotasm1", [P, 16], F32, kind="ExternalInput")
    iotat1_d = nc.dram_tensor("iotat1", [P, TT], F32, kind="ExternalInput")
    iotat_d = nc.dram_tensor("iotat", [P, TT], F32, kind="ExternalInput")
    esel_d = nc.dram_tensor("esel", [P, E], F32, kind="ExternalInput")
    ltri_d = nc.dram_tensor("ltri", [P, P], F32, kind="ExternalInput")
    outsh_d = nc.dram_tensor("outsh", [T, H], BF16, kind="ExternalOutput")
    # scatter-add target, +1 row shifted: row 0 is the trash row; the host
    # slices [1:]
    routed_d = nc.dram_tensor("routed", [T + 1, H], BF16,
                              kind="ExternalOutput")
    # tiny scratch target for the DMA-ordering trick (never read back)
    ordr_d = nc.dram_tensor("ordr", [1, CPAD], BF16, kind="ExternalOutput")
    DBG = os.environ.get("KDBG") == "1"
    if DBG:
        dbg_xe_d = nc.dram_tensor("dbg_xe", [P, HO, CPAD], BF16,
                                  kind="ExternalOutput")
        dbg_cw_d = nc.dram_tensor("dbg_cw", [P, CPAD], BF16,
                                  kind="ExternalOutput")
        dbg_idx_d = nc.dram_tensor("dbg_idx", [P, CPAD // 16], I16,
                                   kind="ExternalOutput")

    with tile.TileContext(nc) as tc:
        with (
            tc.tile_pool(name="persist", bufs=1) as pp,
            tc.tile_pool(name="wstream", bufs=2) as wp,
            tc.tile_pool(name="outst", bufs=3) as op,
            tc.tile_pool(name="ps_big", bufs=2, space="PSUM") as ps_b,
            tc.tile_pool(name="ps_sm", bufs=2, space="PSUM") as ps_s,
        ):
            # ---- constants ----
            ident = pp.tile([P, P], F32, tag="ident", name="ident")
            make_identity(nc, ident)
            identb = pp.tile([P, P], BF16, tag="identb", name="identb")
            nc.vector.tensor_copy(identb, ident)
            onesb = pp.tile([P, P], BF16, tag="onesb", name="onesb")
            nc.vector.memset(onesb, 1.0)
            onescol = pp.tile([P, 1], F32, tag="onescol", name="onescol")
            nc.vector.memset(onescol, 1.0)
            allones8 = pp.tile([TT, P], F32, tag="allones8", name="allones8")
            nc.vector.memset(allones8, 1.0)

            # ---- early DMAs (order sets the HBM bus order) ----
            xtb = pp.tile([P, HO, T], BF16, tag="xtb", name="xtb")
            xtr = pp.tile([P, HO, T], BF16, tag="xtr", name="xtr")
            sg_sb = pp.tile([P, HO, SIS], BF16, tag="sg", name="sg_sb")
            su_sb = pp.tile([P, HO, SIS], BF16, tag="su", name="su_sb")
            rwb = pp.tile([P, HO, E], BF16, tag="rwb", name="rwb")
            rwr = pp.tile([P, HO, E], BF16, tag="rwr", name="rwr")

            def q(n):
                return slice(n * QF, (n + 1) * QF)

            # tensor-engine p-state warmup: dependency-free transposes keep
            # the PE busy through the first DMA latency so real work starts
            # at full clock
            ps_warm = ps_s.tile([P, P], BF16, tag="sm", name="ps_warm")
            for _ in range(35):
                nc.tensor.transpose(ps_warm, identb, identb)
            ps_warm2 = ps_s.tile([P, P], BF16, tag="sm", name="ps_warm2")
            for _ in range(35):
                nc.tensor.transpose(ps_warm2, identb, identb)

            nc.sync.dma_start(xtb[:, :, q(0)], xtb_d[:, :, q(0)])
            nc.sync.dma_start(sg_sb[:, :, 0:256], sgb_d[:, :, 0:256])
            nc.sync.dma_start(su_sb[:, :, 0:256], sub_d[:, :, 0:256])
            nc.sync.dma_start(xtb[:, :, q(1)], xtb_d[:, :, q(1)])
            nc.sync.dma_start(rwb, rwb_d[:])
            nc.sync.dma_start(rwr, rwr_d[:])
            nc.sync.dma_start(xtr[:, :, 0:NF], xtr_d[:, :, 0:NF])
            nc.sync.dma_start(sg_sb[:, :, 256:512], sgb_d[:, :, 256:512])
            nc.sync.dma_start(su_sb[:, :, 256:512], sub_d[:, :, 256:512])
            nc.sync.dma_start(xtb[:, :, q(2)], xtb_d[:, :, q(2)])
            nc.sync.dma_start(xtr[:, :, NF:T], xtr_d[:, :, NF:T])
            nc.sync.dma_start(xtb[:, :, q(3)], xtb_d[:, :, q(3)])

            # small constants
            iotac = pp.tile([P, CPAD], F32, tag="iotac", name="iotac")
            nc.sync.dma_start(iotac, iotac_d[:])
            iotag = pp.tile([P, 16], F32, tag="iotag", name="iotag")
            nc.sync.dma_start(iotag, iotag_d[:])
            iotam = pp.tile([P, P], F32, tag="iotam", name="iotam")
            nc.sync.dma_start(iotam, iotam_d[:])
            iotasm1 = pp.tile([P, 16], F32, tag="iotasm1", name="iotasm1")
            nc.sync.dma_start(iotasm1, iotasm1_d[:])
            iotat1 = pp.tile([P, TT], F32, tag="iotat1", name="iotat1")
            nc.sync.dma_start(iotat1, iotat1_d[:])
            iotat = pp.tile([P, TT], F32, tag="iotat", name="iotat")
            nc.sync.dma_start(iotat, iotat_d[:])
            esel_sb = pp.tile([P, E], F32, tag="esel", name="esel_sb")
            nc.sync.dma_start(esel_sb, esel_d[:])
            ltri = pp.tile([P, P], F32, tag="ltri", name="ltri")
            nc.sync.dma_start(ltri, ltri_d[:])

            # shared-down weights; expert weights are DMA'd after the token
            # gather (their SP dispatch is held behind the ordering DMA so
            # the gather gets an early slot on the DMA engines)
            sd_sb = pp.tile([P, ST, H], BF16, tag="sd", name="sd_sb")
            nc.sync.dma_start(sd_sb, sdb_d[:])

            # ---- helpers ----
            gsT = pp.tile([P, ST, T], BF16, tag="gsT", name="gsT")
            L_sb = pp.tile([P, TT, E], F32, tag="L", name="L_sb")

            ps_q_ctx = tc.tile_pool(name="ps_q", bufs=4, space="PSUM")
            ps_q = ps_q_ctx.__enter__()

            def shared_g(si, qn):
                qsl = q(qn)
                psg = ps_q.tile([P, QF], F32, tag="ps_q", name="psg_s")
                for ko in range(HO):
                    nc.tensor.matmul(psg, sg_sb[:, ko, si * P:(si + 1) * P],
                                     xtb[:, ko, qsl],
                                     start=(ko == 0), stop=(ko == HO - 1))
                return psg

            def shared_u(si, qn, psg):
                qsl = q(qn)
                psu = ps_q.tile([P, QF], F32, tag="ps_q", name="psu_s")
                for ko in range(HO):
                    nc.tensor.matmul(psu, su_sb[:, ko, si * P:(si + 1) * P],
                                     xtb[:, ko, qsl],
                                     start=(ko == 0), stop=(ko == HO - 1))
                sil = op.tile([P, QF], BF16, tag="sil", name="sil_s", bufs=2)
                nc.scalar.activation(sil, psg, AF.Silu)
                nc.vector.tensor_tensor(gsT[:, si, qsl], sil, psu, ALU.mult)

            def shared_gu(si, qn):
                shared_u(si, qn, shared_g(si, qn))

            def router(tt):
                psL = ps_s.tile([P, E], F32, tag="sm", name="psL")
                tsl = slice(tt * P, (tt + 1) * P)
                k = 0
                for (xs, ws) in ((xtb, rwb), (xtb, rwr), (xtr, rwb)):
                    for ko in range(HO):
                        nc.tensor.matmul(psL, xs[:, ko, tsl], ws[:, ko, :],
                                         start=(k == 0), stop=(k == 3 * HO - 1))
                        k += 1
                nc.vector.tensor_copy(L_sb[:, tt, :], psL)

            # ---- phase 1: shared g/u on q0/q1 for si0/si1, then router ----
            shared_gu(0, 0)
            shared_gu(1, 0)
            shared_gu(0, 1)
            shared_gu(1, 1)
            for tt in range(4):
                router(tt)
            # g for both si first: the su slab lands ~1.5us after the sg slab
            psg20 = shared_g(2, 0)
            psg30 = shared_g(3, 0)
            shared_u(2, 0, psg20)
            shared_u(3, 0, psg30)
            for tt in range(4, 8):
                router(tt)

            # ---- top-1 combine: mask m_sb and weight combw, both [t_p, tt] --
            maxc = pp.tile([P, TT], F32, tag="maxc", name="maxc")
            nc.vector.reduce_max(maxc, L_sb, axis=AX.X)
            w_sb = pp.tile([P, TT], F32, tag="wsb", name="w_sb")
            nc.scalar.activation(w_sb, maxc, AF.Sigmoid)
            eq = pp.tile([P, TT, E], F32, tag="eq", name="eq")
            nc.vector.tensor_tensor(eq, L_sb,
                                    maxc[:, :, None].to_broadcast([P, TT, E]),
                                    ALU.is_equal)
            nc.vector.tensor_tensor(eq, eq,
                                    esel_sb[:, None, :].to_broadcast([P, TT, E]),
                                    ALU.mult)
            m_sb = pp.tile([P, TT], F32, tag="m", name="m_sb")
            nc.vector.reduce_sum(m_sb, eq, axis=AX.X)
            combw = pp.tile([P, TT], F32, tag="combw", name="combw")
            nc.vector.tensor_tensor(combw, m_sb, w_sb, ALU.mult)

            shared_gu(0, 2)
            shared_gu(1, 2)

            # ---- capacity slots: slot[t] = #selected tokens before t ----
            ps_cs = ps_s.tile([P, TT], F32, tag="sm", name="ps_cs")
            nc.tensor.matmul(ps_cs, ltri, m_sb, start=True, stop=True)
            ps_sm2 = ps_s.tile([TT, 1], F32, tag="sm", name="ps_sm2")
            nc.tensor.matmul(ps_sm2, m_sb, onescol, start=True, stop=True)
            sumsT = pp.tile([TT, 1], F32, tag="sumsT", name="sumsT")
            nc.vector.tensor_copy(sumsT, ps_sm2)
            LS = pp.tile([TT, TT], F32, tag="LS", name="LS")
            nc.vector.tensor_tensor(LS, ltri[:TT, :TT],
                                    sumsT.to_broadcast([TT, TT]), ALU.mult)
            slot = pp.tile([P, TT], F32, tag="slot", name="slot")
            nc.vector.tensor_copy(slot, ps_cs)
            ps_off = ps_s.tile([P, TT], F32, tag="sm", name="ps_off")
            nc.tensor.matmul(ps_off, allones8, LS, start=True, stop=True)

            nc.vector.tensor_tensor(slot, slot, ps_off, ALU.add)
            slotm = pp.tile([P, TT], F32, tag="slotm", name="slotm")
            nc.vector.tensor_tensor(slotm, slot, m_sb, ALU.mult)
            inv = pp.tile([P, TT], F32, tag="inv", name="inv")
            nc.vector.tensor_scalar(inv, m_sb, -BIG, BIG, ALU.mult, ALU.add)
            nc.vector.tensor_tensor(slotm, slotm, inv, ALU.add)

            # ---- wrapped gather/scatter index lists (int16) ----
            # idx[j] lives at [j%16, j//16]; factor (slot==j) as
            # (slot%16 == p%16) x (slot//16 == j//16); the p%16 form makes
            # the matmul output stripe-replicated for the 8 Q7 cores.
            div16 = pp.tile([P, TT], F32, tag="div16", name="div16")
            tmp16 = pp.tile([P, TT, 16], F32, tag="tmp16", name="tmp16")
            nc.vector.tensor_tensor(tmp16,
                                    slotm[:, :, None].to_broadcast([P, TT, 16]),
                                    iotag[:, None, :].to_broadcast([P, TT, 16]),
                                    ALU.is_ge)
            nc.vector.reduce_sum(div16, tmp16, axis=AX.X)
            mod16 = pp.tile([P, TT], F32, tag="mod16", name="mod16")
            nc.vector.tensor_scalar(mod16, div16, -16.0, 0.0, ALU.mult,
                                    ALU.add)
            nc.vector.tensor_tensor(mod16, slotm, mod16, ALU.add)
            lhs16 = pp.tile([P, TT, P], F32, tag="lhs16", name="lhs16")
            nc.vector.tensor_tensor(lhs16,
                                    mod16[:, :, None].to_broadcast([P, TT, P]),
                                    iotam[:, None, :].to_broadcast([P, TT, P]),
                                    ALU.is_equal)
            rhsI = pp.tile([P, TT, 16], F32, tag="rhsI", name="rhsI")
            nc.vector.tensor_tensor(rhsI,
                                    div16[:, :, None].to_broadcast([P, TT, 16]),
                                    iotac[:, None, :16].to_broadcast([P, TT, 16]),
                                    ALU.is_equal)
            nc.vector.tensor_tensor(rhsI, rhsI,
                                    iotat[:, :, None].to_broadcast([P, TT, 16]),
                                    ALU.mult)
            # scatter list: entry 16+j -> row tok[j]+1 (entry-0 drop +
            # trash row 0). slot+16 keeps mod16; div16+1 shifts the column.
            rhsIp = pp.tile([P, TT, 16], F32, tag="rhsIp", name="rhsIp")
            nc.vector.tensor_tensor(rhsIp,
                                    div16[:, :, None].to_broadcast([P, TT, 16]),
                                    iotasm1[:, None, :].to_broadcast([P, TT, 16]),
                                    ALU.is_equal)
            nc.vector.tensor_tensor(rhsIp, rhsIp,
                                    iotat1[:, :, None].to_broadcast([P, TT, 16]),
                                    ALU.mult)
            # per-slot combine weight row (perm carries combw)
            perm = pp.tile([P, TT, C], BF16, tag="perm", name="perm")
            nc.vector.tensor_tensor(
                perm, slotm[:, :, None].to_broadcast([P, TT, C]),
                iotac[:, None, :C].to_broadcast([P, TT, C]), ALU.is_equal)
            nc.vector.tensor_tensor(
                perm, perm, combw[:, :, None].to_broadcast([P, TT, C]),
                ALU.mult)

            shared_gu(2, 1)

            ps_idx = ps_s.tile([P, 16], F32, tag="sm", name="ps_idx")
            for tt in range(TT):
                nc.tensor.matmul(ps_idx, lhs16[:, tt, :], rhsI[:, tt, :],
                                 start=(tt == 0), stop=(tt == TT - 1))
            idx16 = pp.tile([P, CPAD // 16], I16, tag="idx16", name="idx16")
            nc.vector.tensor_copy(idx16, ps_idx)

            # ---- token gather (DMA, transposed): xeT[p, ho, j] ----
            xeT = pp.tile([P, HO, CPAD], BF16, tag="xeT", name="xeT")
            nc.gpsimd.dma_gather(xeT[:], xnat_d[:], idx16[:], CPAD, CPAD, H,
                                 transpose=True)

            # ordering DMA: reads xeT, so every weight DMA emitted after it
            # dispatches (and requests the DMA engines) after the gather
            nc.sync.dma_start(ordr_d[:], xeT[0:1, 0, :])
            eg_tiles, eu_tiles, ed_tiles = [], [], []
            for ib in range(4):
                eg_sl = wp.tile([P, HO, 512], BF16, tag="eg", name="eg_sl",
                                bufs=4)
                nc.sync.dma_start(eg_sl, egb_d[:, :, ib * 512:(ib + 1) * 512])
                eu_sl = wp.tile([P, HO, 512], BF16, tag="eu", name="eu_sl",
                                bufs=4)
                nc.sync.dma_start(eu_sl, eub_d[:, :, ib * 512:(ib + 1) * 512])
                eg_tiles.append(eg_sl)
                eu_tiles.append(eu_sl)
            for hb in range(4):
                ed_sl = wp.tile([P, IT, 256], BF16, tag="ed", name="ed_sl",
                                bufs=4)
                nc.sync.dma_start(ed_sl, edb_d[:, :, hb * 256:(hb + 1) * 256])
                ed_tiles.append(ed_sl)

            shared_gu(3, 1)

            ps_cw = ps_s.tile([P, C], F32, tag="sm", name="ps_cw")
            for tt in range(TT):
                nc.tensor.matmul(ps_cw, onesb, perm[:, tt, :],
                                 start=(tt == 0), stop=(tt == TT - 1))
            cwB = pp.tile([P, C], BF16, tag="cwB", name="cwB")
            nc.vector.tensor_copy(cwB, ps_cw)
            # scale gathered tokens by their routing weight on the (idle)
            # gpsimd engine, right behind the gather on the Pool queue
            xeTs = pp.tile([P, HO, C], BF16, tag="xeTs", name="xeTs")
            for ko in range(HO):
                nc.gpsimd.tensor_tensor(xeTs[:, ko, :], xeT[:, ko, 0:C], cwB,
                                        ALU.mult)

            ps_idx2 = ps_s.tile([P, 16], F32, tag="sm", name="ps_idx2")
            for tt in range(TT):
                nc.tensor.matmul(ps_idx2, lhs16[:, tt, :], rhsIp[:, tt, :],
                                 start=(tt == 0), stop=(tt == TT - 1))
            idx16s = pp.tile([P, CPAD // 16], I16, tag="idx16s",
                             name="idx16s")
            nc.vector.tensor_copy(idx16s, ps_idx2)

            # remaining shared quarters
            shared_gu(0, 3)
            shared_gu(1, 3)
            shared_gu(2, 2)
            shared_gu(3, 2)
            shared_gu(2, 3)
            shared_gu(3, 3)
            ps_q_ctx.__exit__(None, None, None)
            ps_g_ctx = tc.tile_pool(name="ps_gu", bufs=4, space="PSUM")
            ps_g = ps_g_ctx.__enter__()

            # ---- shared down (tokens on partitions) + out partial ----
            def shared_down(tt):
                o_t = op.tile([P, H], BF16, tag="ot", name="o_t")
                for hh in range(NH):
                    psD = ps_b.tile([P, NF], F32, tag="ps_b", name="psD")
                    for sk in range(ST):
                        nc.tensor.matmul(psD,
                                         gsT[:, sk, tt * P:(tt + 1) * P],
                                         sd_sb[:, sk, hh * NF:(hh + 1) * NF],
                                         start=(sk == 0), stop=(sk == ST - 1))
                    hsl = slice(hh * NF, (hh + 1) * NF)
                    # copy + DMA issue on the same engine: the two halves'
                    # DMA dispatches overlap instead of serializing on SP
                    if hh == 0:
                        nc.scalar.activation(o_t[:, hsl], psD, AF.Copy)
                        nc.scalar.dma_start(outsh_d[tt * P:(tt + 1) * P, hsl],
                                            o_t[:, hsl])
                    else:
                        nc.vector.tensor_copy(o_t[:, hsl], psD)
                        nc.sync.dma_start(outsh_d[tt * P:(tt + 1) * P, hsl],
                                          o_t[:, hsl])

            for tt in range(4):
                shared_down(tt)

            if DBG:
                nc.sync.dma_start(dbg_xe_d[:], xeT)
                nc.sync.dma_start(dbg_cw_d[:, 0:C], cwB)
                nc.sync.dma_start(dbg_idx_d[:], idx16)

            # ---- expert gate/up at capacity C -> gTe[i_p, it, j] ----
            gTe = pp.tile([P, IT, C], BF16, tag="gTe", name="gTe")
            for ib in range(4):
                for a in range(4):
                    it = ib * 4 + a
                    psg = ps_g.tile([P, C], F32, tag="ps_g", name="psg_e")
                    for ko in range(HO):
                        nc.tensor.matmul(psg,
                                         eg_tiles[ib][:, ko,
                                                      a * P:(a + 1) * P],
                                         xeTs[:, ko, :],
                                         start=(ko == 0), stop=(ko == HO - 1))
                    psu = ps_g.tile([P, C], F32, tag="ps_g", name="psu_e")
                    for ko in range(HO):
                        nc.tensor.matmul(psu,
                                         eu_tiles[ib][:, ko,
                                                      a * P:(a + 1) * P],
                                         xeTs[:, ko, :],
                                         start=(ko == 0), stop=(ko == HO - 1))
                    sil = op.tile([P, C], BF16, tag="sil_e", name="sil_e",
                                  bufs=2)
                    nc.scalar.activation(sil, psg, AF.Silu)
                    nc.vector.tensor_tensor(gTe[:, it, :], sil, psu, ALU.mult)

            # ---- expert down [h_p, j], transpose to entry rows, scatter ----
            # software-pipelined: transposes of ho run under the next ho's
            # matmuls so the PE never waits on the re_sb copy latency
            scT = pp.tile([P, CB, H], BF16, tag="scT", name="scT")

            def down_mm(ho):
                hb, hj = ho // 2, ho % 2
                psd = ps_g.tile([P, C], F32, tag="ps_g", name="psd")
                for ik in range(IT):
                    nc.tensor.matmul(psd,
                                     ed_tiles[hb][:, ik,
                                                  hj * P:(hj + 1) * P],
                                     gTe[:, ik, :],
                                     start=(ik == 0), stop=(ik == IT - 1))
                re_sb = op.tile([P, CE], BF16, tag="re", name="re_sb")
                nc.vector.memset(re_sb[:, 0:16], 0.0)
                nc.scalar.activation(re_sb[:, 16:16 + C], psd, AF.Copy)
                return re_sb

            def down_tr(ho, re_sb):
                for cb in range(CB):
                    cq = min(P, CE - cb * P)
                    ps_tr = ps_s.tile([P, P], BF16, tag="sm", name="ps_tr")
                    nc.tensor.transpose(ps_tr[0:cq, :],
                                        re_sb[:, cb * P:cb * P + cq],
                                        identb)
                    nc.scalar.activation(
                        scT[0:cq, cb, ho * P:(ho + 1) * P],
                        ps_tr[0:cq, :], AF.Copy)

            prev = down_mm(0)
            for ho in range(1, HO):
                cur = down_mm(ho)
                down_tr(ho - 1, prev)
                prev = cur
            down_tr(HO - 1, prev)

            nc.gpsimd.dma_scatter_add(routed_d[:], scT[:], idx16s[:, 0:CI],
                                      CE, CE, H)
            ps_g_ctx.__exit__(None, None, None)

            for tt in range(4, 8):
                shared_down(tt)

    nc.compile()
    return nc


@functools.lru_cache(maxsize=2)
def _get_nc_for(C):
    return _build_nc(C)


_LAST_NC = None


def _get_nc():
    return _LAST_NC


def _pick_capacity(x, rw):
    logits = x.astype(np.float32) @ rw.astype(np.float32).T
    loads = np.bincount(logits.argmax(1), minlength=E)
    c = int(loads.max()) + 8
    c = (c + 15) // 16 * 16
    return max(32, min(CPAD - 16, c))


def _rearr(w, nblk):
    # [(k p), cols] -> [p, k, cols]
    return np.ascontiguousarray(
        w.reshape(nblk, P, -1).transpose(1, 0, 2))


def _make_in_maps(inputs):
    f32 = lambda v: np.asarray(v, dtype=np.float32)
    x = f32(inputs["hidden_states"])
    rw = f32(inputs["router_weight"])
    sg = f32(inputs["shared_gate"])
    su = f32(inputs["shared_up"])
    sd = f32(inputs["shared_down"])
    eg = f32(inputs["expert_gate"])
    eu = f32(inputs["expert_up"])
    ed = f32(inputs["expert_down"])

    xT = np.ascontiguousarray(x.T)                    # [H, T]
    xTb = xT.astype(BF)
    xTr = (xT - xTb.astype(np.float32)).astype(BF)
    rwT = np.ascontiguousarray(rw.T)                  # [H, E]
    rwTb = rwT.astype(BF)
    rwTr = (rwT - rwTb.astype(np.float32)).astype(BF)

    iotac = np.tile(np.arange(CPAD, dtype=np.float32), (P, 1))
    iotag = np.tile(16.0 * np.arange(1, 17, dtype=np.float32), (P, 1))
    iotam = np.tile((np.arange(P) % 16).astype(np.float32), (P, 1))
    iotasm1 = np.tile(np.arange(-1, 15, dtype=np.float32), (P, 1))
    iotat = (np.arange(P, dtype=np.float32)[:, None]
             + P * np.arange(TT, dtype=np.float32)[None, :])
    ltri = np.triu(np.ones((P, P), dtype=np.float32), 1)

    common = {
        "xtb": _rearr(xTb, HO),
        "xtr": _rearr(xTr, HO),
        "xnat": np.ascontiguousarray(x.astype(BF)),
        "rwb": _rearr(rwTb, HO),
        "rwr": _rearr(rwTr, HO),
        "iotac": iotac,
        "iotag": np.ascontiguousarray(iotag),
        "iotam": np.ascontiguousarray(iotam),
        "iotasm1": np.ascontiguousarray(iotasm1),
        "iotat1": np.ascontiguousarray(iotat + 1.0),
        "iotat": np.ascontiguousarray(iotat),
        "ltri": ltri,
    }
    in_maps = []
    for c in range(NCORES):
        esel = np.zeros((P, E), dtype=np.float32)
        esel[:, c] = 1.0
        sl = slice(c * SIS, (c + 1) * SIS)
        in_maps.append({
            **common,
            "esel": esel,
            "sgb": _rearr(sg[:, sl].astype(BF), HO),
            "sub": _rearr(su[:, sl].astype(BF), HO),
            "sdb": _rearr(sd[sl, :].astype(BF), ST),
            "egb": _rearr(eg[c].astype(BF), HO),
            "eub": _rearr(eu[c].astype(BF), HO),
            "edb": _rearr(ed[c].astype(BF), IT),
        })
    return in_maps


def _run(inputs, trace=False):
    global _LAST_NC
    from concourse.bass_utils import run_bass_kernel_spmd
    C = _pick_capacity(np.asarray(inputs["hidden_states"]),
                       np.asarray(inputs["router_weight"]))
    nc = _get_nc_for(C)
    _LAST_NC = nc
    in_maps = _make_in_maps(inputs)
    res = run_bass_kernel_spmd(nc, in_maps, core_ids=list(range(NCORES)),
                               trace=trace)
    acc = np.zeros((T, H), dtype=np.float32)
    for r in res.results:
        acc += r["outsh"].astype(np.float32)
        acc += r["routed"][1:].astype(np.float32)
    return acc, res


def kernel(**inputs) -> np.ndarray:
    out, _ = _run(inputs, trace=False)
    return out
